# revision 56
# baseline (speedup 1.0000x reference)
# Multi-head causal self-attention (B=2, S=2048, D=768, H=12) on 8 NeuronCores.
#
# Sharding: (batch, head-group) across cores. Core c handles batch c//4 and
# heads 3*(c%4) .. 3*(c%4)+2. Each core computes its heads' Q/K/V projections
# (column-sharded), the causal attention for those heads, and a row-sharded
# partial of the output projection. Host sums the 4 partials per batch + bo.
#
# Engine plan (v5):
#  - PE batches work by tile-size mode so the array never mode-switch-drains
#    mid-stream: (128,128) for QK/V projections and AV (M=65, ones column
#    accumulates the softmax denominator); (64,128) for scores (heads 0,1
#    pair-stacked on partitions and issued to row tiles T0/T8 which run
#    concurrently; head 2 on T0) and the K=64 output projection.
#  - Attention chunks are processed in DESCENDING size order (ic3..ic0) with
#    all QK projections hoisted to the front: the long exp streams start
#    early and the kernel tail is the smallest chunk.
#  - ACT runs exp and the z-chain. 1/Z = exp(-ln Z), entirely on-engine
#    (no DMA round trips): ACT Ln reads the Z row straight from PSUM, a
#    K=1 fp32 matmul broadcasts ln Z into the AV tile's unused partitions
#    64:128, ACT Exp(scale=-1) evacuates 1/Z to SBUF, DVE multiplies.
#    (ACT Reciprocal is blocked in bass for accuracy; Ln+Exp is accurate.)
#  - Initial loads are batched into few triggers spread over 4 engine
#    queues so the first projection data lands ~6us in; PE ramps HAM to
#    full duty on real work instead of idling on DMA dispatch.
#  - GPSIMD: causal-mask multiplies. Output DMA'd in bf16; host sums in fp32.

import sys

import ml_dtypes
import numpy as np

sys.path.insert(0, "/opt/trn_rl_repo")

import concourse.bass as bass  # noqa: E402
import concourse.mybir as mybir  # noqa: E402
import concourse.tile as tile  # noqa: E402
from concourse.bass import ts  # noqa: E402
from concourse.bass_utils import run_bass_kernel_spmd  # noqa: E402

F32 = mybir.dt.float32
BF16 = mybir.dt.bfloat16
AF = mybir.ActivationFunctionType
MUL = mybir.AluOpType.mult
ADD = mybir.AluOpType.add
NPBF16 = ml_dtypes.bfloat16

B, S, D, H, HD = 2, 2048, 768, 12, 64
HPC = 3               # heads per core
DQK = 2 * HPC * HD    # 384
DV = HPC * HD         # 192
P = 128
IC = S // 512         # 4 query chunks of 512
KC = D // P           # 6 contraction chunks
NIO = S // P          # 16 token chunks of 128


def _split_excess_waits(nc, max_waits=1):
    # walrus in this env rejects instructions carrying more than ~1-2
    # sync-waits. Move excess waits onto preceding same-engine nops.
    n_split = 0
    for func in nc.m.functions:
        for blk in func.blocks:
            insts = blk.instructions
            out = []
            changed = False
            for inst in insts:
                si = inst.sync_info
                waits = list(si.on_wait) if si and si.on_wait else []
                if len(waits) > max_waits:
                    changed = True
                    for j, w in enumerate(waits[:-max_waits]):
                        out.append(
                            mybir.InstNoOp(
                                name=f"{inst.name}-wsplit{j}",
                                engine=inst.engine,
                                ins=[],
                                outs=[],
                                sync_info=mybir.SyncInfo(
                                    on_wait=[w], on_update=[]
                                ),
                            )
                        )
                        n_split += 1
                    inst.sync_info = mybir.SyncInfo(
                        on_wait=waits[-max_waits:],
                        on_update=list(si.on_update) if si.on_update else [],
                    )
                out.append(inst)
            if changed:
                blk.instructions = out
    return n_split


def _build_module():
    # All weight/activation DRAM tensors are host-pre-rearranged so that
    # every SBUF load is one descriptor per partition (contiguous src and
    # dst): descriptor generation is ~10ns/descriptor, so 128 fat
    # descriptors start flowing ~7us earlier than 768 thin ones.
    nc = bass.Bass()
    xt_d = nc.dram_tensor("xt", [P, IC, KC, 512], BF16, kind="ExternalInput")
    wqk_d = nc.dram_tensor("wqk", [P, HPC, KC, P], BF16, kind="ExternalInput")
    bqk_d = nc.dram_tensor("bqk", [P, HPC], F32, kind="ExternalInput")
    wv_d = nc.dram_tensor("wv", [P, KC, DV], BF16, kind="ExternalInput")
    wos_d = nc.dram_tensor("wos", [P, 2, D], BF16, kind="ExternalInput")
    mask_d = nc.dram_tensor("mask", [P, 4, 2, 512], BF16, kind="ExternalInput")
    out_d = nc.dram_tensor("out", [S, D], BF16, kind="ExternalOutput")
    scratch_d = nc.dram_tensor("scratch", [HD + 1, 512], F32)
    gate_d = nc.dram_tensor("gatescr", [1, 8], BF16)

    with tile.TileContext(nc) as tc:
        with (
            tc.tile_pool(name="const", bufs=1) as cp,
            tc.tile_pool(name="exp", bufs=40) as exp_p,
            tc.tile_pool(name="zr", bufs=2) as zr_p,
            tc.tile_pool(name="outp", bufs=2) as op,
            tc.tile_pool(name="proj", bufs=2, space="PSUM") as proj_p,
            tc.tile_pool(name="scps", bufs=2, space="PSUM") as sc_p,
            tc.tile_pool(name="avps", bufs=2, space="PSUM") as av_p,
        ):
            # ---- PE warm-up source via DVE memset (gpsimd starts slowly) ----
            warm_src = cp.tile([P, 520], BF16)
            nc.vector.memset(warm_src, 1.0)

            # ---- resident SBUF tensors ----
            # The 16 hw DMA engines are a shared ~250GB/s pool: concurrent
            # transfers steal bandwidth from each other, so the critical
            # first-projection loads (xt chunk 0 + wqk) are issued alone;
            # the bulk (mask/wv/wos, xt chunks 1+2) is gated behind tiny
            # compute ops that only unblock once the first projections are
            # evacuating (~13us), keeping the early window clean.
            wqk_sb = cp.tile([P, HPC, KC, P], BF16)
            xt_sb = cp.tile([P, IC, KC, 512], BF16)
            bqk_sb = cp.tile([P, HPC], F32)
            wv_sb = cp.tile([P, KC, DV], BF16)
            mask_sb = cp.tile([P, 4, 2, 512], BF16)
            wos_sb = cp.tile([P, 2, D], BF16)

            # The DMA engines round-robin among ALL active transfers
            # (~250GB/s shared, data flow starts ~8.5us in), so issue the
            # early-needed set together and gate only the genuinely
            # late-needed bulk (xt chunk 2, mask, wv, wos) behind compute.
            nc.sync.dma_start(wqk_sb[:, 0], wqk_d[:, 0])
            nc.sync.dma_start(xt_sb[:, 0], xt_d[:, 0])
            nc.sync.dma_start(wqk_sb[:, 1:3], wqk_d[:, 1:3])
            nc.sync.dma_start(xt_sb[:, 3], xt_d[:, 3])
            nc.sync.dma_start(bqk_sb, bqk_d[:])
            nc.sync.dma_start(xt_sb[:, 1], xt_d[:, 1])

            # V with ones columns HD:P: every AV psum row 64:128 then
            # accumulates the softmax denominator Z, pre-broadcast, and the
            # AV matmul gets a full 128-wide stationary operand. Cols 0:HD
            # are overwritten by the V-projection evacuations.
            v1 = cp.tile([P, NIO, HPC, P], BF16)
            nc.vector.memset(v1[:, :, :, HD:P], 1.0)

            # pair-stacked Q^T/K^T for heads 0,1; head 2's K/Q duplicated
            # into both partition halves so its scores pair on T0/T8 too
            qTp = cp.tile([P, S], BF16)
            klp = cp.tile([P, S], BF16)
            qT2d = cp.tile([P, S], BF16)
            kl2d = cp.tile([P, S], BF16)
            # ctx for h0/h1 stacked on partitions: the output projection
            # contracts both heads in one K=128 matmul; h2 separate (K=64)
            ctxT01 = cp.tile([P, S], BF16)
            ctxT2 = cp.tile([HD, S], BF16)

            # ---- PE warm-up: (128,128)-mode matmuls ----
            # Just enough to bridge until the first projection's DMA lands
            # (~6us); the projections themselves sustain the HAM ramp.
            warm_ps = av_p.tile([P, 512], F32, tag="av", name="warm")
            for w in range(5):
                nc.tensor.matmul(
                    warm_ps[0 : HD + 1, :],
                    lhsT=warm_src[:, 0:65],
                    rhs=warm_src[:, 0:512],
                    start=True,
                    stop=(w == 4),
                )
            warm_sb = zr_p.tile([HD + 1, 512], F32, tag="warm", name="warmsb")
            nc.vector.tensor_copy(warm_sb, warm_ps[0 : HD + 1, :])
            nc.sync.dma_start(scratch_d[:], warm_sb)

            carry = {}   # ic -> (pair ex tiles, h2 ex tiles)

            def proj_slice(ic, sl):
                # wqk slices: 0 -> [K_h0|K_h1], 1 -> [K_h2|Q_h2],
                #             2 -> [Q_h0|Q_h1]
                isl = ts(ic, 512)
                ps = proj_p.tile([P, 512], F32, tag="proj")
                for kc in range(KC):
                    nc.tensor.matmul(
                        ps,
                        lhsT=wqk_sb[:, sl, kc, :],
                        rhs=xt_sb[:, ic, kc, :],
                        start=(kc == 0),
                        stop=(kc == KC - 1),
                    )
                if sl == 0:
                    nc.vector.tensor_scalar(
                        klp[:, isl], ps, bqk_sb[:, 0:1], None, ADD,
                    )
                elif sl == 1:
                    # duplicate K_h2/Q_h2 into both partition halves
                    nc.vector.tensor_scalar(
                        kl2d[0:HD, isl], ps[0:HD, :], bqk_sb[0:HD, 1:2],
                        None, ADD,
                    )
                    nc.vector.tensor_scalar(
                        kl2d[HD:P, isl], ps[0:HD, :], bqk_sb[0:HD, 1:2],
                        None, ADD,
                    )
                    nc.vector.tensor_scalar(
                        qT2d[0:HD, isl], ps[HD:P, :], bqk_sb[HD:P, 1:2],
                        None, ADD,
                    )
                    nc.vector.tensor_scalar(
                        qT2d[HD:P, isl], ps[HD:P, :], bqk_sb[HD:P, 1:2],
                        None, ADD,
                    )
                else:
                    nc.vector.tensor_scalar(
                        qTp[:, isl], ps, bqk_sb[:, 2:3], None, ADD,
                    )

            def trim_of(jc, ic):
                koff = jc - 4 * ic
                return P * koff if koff > 0 else 0

            def sc_group_pair(ic, jc):
                t = trim_of(jc, ic)
                koff = jc - 4 * ic
                sc = sc_p.tile([P, 2, 512], F32, tag="sc", name=f"sp{ic}_{jc}")
                for h in range(2):
                    hsl = ts(h, HD)
                    nc.tensor.matmul(
                        sc[:, h, t:],
                        lhsT=klp[hsl, ts(jc, P)],
                        rhs=qTp[hsl, ic * 512 + t : (ic + 1) * 512],
                        start=True,
                        stop=True,
                    )
                ex = exp_p.tile([P, 2, 512], BF16, tag="ex", name=f"xp{ic}_{jc}")
                nc.scalar.activation(ex[:, :, t:], sc[:, :, t:], AF.Exp)
                if koff >= 0:
                    nc.gpsimd.tensor_tensor(
                        ex[:, :, t:], ex[:, :, t:],
                        mask_sb[:, koff, :, t:], MUL,
                    )
                carry[ic][0].append(ex)

            def sc_group_h2(ic, jb):
                # the two key-blocks go to T0/T8 concurrently via the
                # duplicated partition halves of kl2d/qT2d
                sc = sc_p.tile([P, 2, 512], F32, tag="sc", name=f"s2_{ic}_{jb}")
                for k in range(2):
                    jc = jb + k
                    t = trim_of(jc, ic)
                    hs = slice(k * HD, (k + 1) * HD)
                    nc.tensor.matmul(
                        sc[:, k, t:],
                        lhsT=kl2d[hs, ts(jc, P)],
                        rhs=qT2d[hs, ic * 512 + t : (ic + 1) * 512],
                        start=True,
                        stop=True,
                    )
                ex = exp_p.tile([P, 2, 512], BF16, tag="ex", name=f"x2_{ic}_{jb}")
                koff = jb - 4 * ic
                if koff >= 0 and trim_of(jb + 1, ic) > 0:
                    for k in range(2):
                        t = trim_of(jb + k, ic)
                        nc.scalar.activation(ex[:, k, t:], sc[:, k, t:], AF.Exp)
                        nc.gpsimd.tensor_tensor(
                            ex[:, k, t:], ex[:, k, t:],
                            mask_sb[:, koff + k, 0, t:], MUL,
                        )
                else:
                    nc.scalar.activation(ex, sc, AF.Exp)
                    if koff >= 0:
                        nc.gpsimd.tensor_tensor(
                            ex, ex, mask_sb[:, koff : koff + 2, 0, :], MUL,
                        )
                carry[ic][1].append(ex)

            # z chains, fully on ACT: the AV psum rows HD:P already hold Z
            # pre-broadcast (ones columns of v1), so 1/Z = exp(-ln Z) is
            # two partition-parallel ACT table ops, no DMA, no PE.
            # (custom-DVE reciprocal_approx_fast would be cheaper still but
            # the axon compile path can't emit custom DVE ISA ops.)
            zbs = {}    # (ic, h) -> zb tile

            def make_zchain_a(ic, h, state):
                def go():
                    lnb = zr_p.tile([HD, 512], F32, tag="lnz",
                                    name=f"ln{ic}{h}")
                    nc.scalar.activation(lnb, state[h][HD:P, :], AF.Ln)
                    zbs[(ic, h, "ln")] = lnb
                return go

            def make_zchain_b(ic, h, state):
                def go():
                    lnb = zbs.pop((ic, h, "ln"))
                    zb = zr_p.tile([HD, 512], F32, tag="zb", name=f"zb{ic}{h}")
                    nc.scalar.activation(zb, lnb, AF.Exp, scale=-1.0)
                    zbs[(ic, h)] = zb
                return go

            def make_mult(ic, h, state, parts=1):
                def go():
                    avt = state.pop(h)
                    zb = zbs.pop((ic, h))
                    w = 512 // parts
                    for c in range(parts):
                        cs = slice(c * w, (c + 1) * w)
                        osl = slice(ic * 512 + c * w, ic * 512 + (c + 1) * w)
                        if h == 0:
                            out = ctxT01[0:HD, osl]
                        elif h == 1:
                            out = ctxT01[HD:P, osl]
                        else:
                            out = ctxT2[:, osl]
                        nc.vector.tensor_tensor(
                            out, avt[0:HD, cs], zb[:, cs], MUL,
                        )
                return go

            def av_stream(ic):
                # mm batches + fin, with each head's mult deferred one head
                n_j = 4 * ic + 4
                state = {}
                work = []

                def mk_mm(h, j0, j1):
                    def go():
                        if h not in state:
                            if ic == 0 and h == 2:
                                t_ = sc_p.tile([P, 2, 512], F32, tag="sc",
                                               name=f"av{ic}{h}")
                                state[h] = t_[:, 0, :]
                            else:
                                state[h] = av_p.tile(
                                    [P, 512], F32, tag="av", name=f"av{ic}{h}"
                                )
                        avt = state[h]
                        exs, exs2 = carry[ic]
                        for jc in range(j0, j1):
                            t = trim_of(jc, ic)
                            exap = (exs[jc][:, h, t:] if h < 2
                                    else exs2[jc // 2][:, jc % 2, t:])
                            nc.tensor.matmul(
                                avt[:, t:],
                                lhsT=v1[:, jc, h, :],
                                rhs=exap,
                                start=(jc == 0),
                                stop=(jc == n_j - 1),
                            )
                    return go

                if ic == 0:
                    # tail chunk: h2's AV accumulates in a free sc-pool bank,
                    # so all three AVs and z-chains overlap before any mult.
                    # Mults split 2x256 so oproj(0) unblocks per half-chunk.
                    for h in range(HPC):
                        for j0 in range(0, n_j, 2):
                            work.append(mk_mm(h, j0, min(j0 + 2, n_j)))
                        work.append(make_zchain_a(ic, h, state))
                        work.append(make_zchain_b(ic, h, state))
                    for h in range(HPC):
                        work.append(make_mult(ic, h, state, parts=2))
                    return work
                for h in range(HPC):
                    # batches of 2 key-chunks with a dummy before every
                    # batch: in the ACT-bound phases the PE inevitably
                    # waits on the live exp stream, and the dummies turn
                    # that wait into HAM-visible activity (idle epochs
                    # halve the duty cycle for 10-30us).
                    mms = [mk_mm(h, j0, min(j0 + 2, n_j))
                           for j0 in range(0, n_j, 2)]
                    items = []
                    for i, m in enumerate(mms):
                        if i >= len(mms) - 2:
                            items.append(dummy128)
                        items.append(m)
                    if h >= 1:
                        items.insert(1, make_zchain_b(ic, h - 1, state))
                        items.insert(2, make_mult(ic, h - 1, state))
                    items.append(make_zchain_a(ic, h, state))
                    work.extend(items)
                work.append(make_zchain_b(ic, HPC - 1, state))
                work.append(make_mult(ic, HPC - 1, state))
                return work

            out_r = out_d.rearrange("(io p) d -> p io d", p=P)
            opair = {}

            def oproj_units(ic, finegrain=False):
                # units in io pairs sharing one o_sb tile and one output DMA;
                # finegrain (tail chunk): one DMA per io on rotating queues
                # so the last write isn't one big serialized transfer.
                units = []
                dma_engines = (nc.sync, nc.gpsimd, nc.scalar, nc.sync)
                for io4 in range(4):
                    io = ic * 4 + io4

                    def unit(io=io, io4=io4):
                        if io % 2 == 0:
                            opair[io // 2] = op.tile(
                                [P, 2, D], BF16, tag="osb", name=f"ou{io}"
                            )
                        o_sb = opair[io // 2]
                        for ot, ow in ((0, 512), (1, 256)):
                            ps = proj_p.tile([P, 512], F32, tag="proj")
                            pso = ps[:, :ow]
                            osl = slice(ot * 512, ot * 512 + ow)
                            nc.tensor.matmul(
                                pso, lhsT=ctxT01[:, ts(io, P)],
                                rhs=wos_sb[:, 0, osl],
                                start=True, stop=False,
                            )
                            nc.tensor.matmul(
                                pso, lhsT=ctxT2[:, ts(io, P)],
                                rhs=wos_sb[0:HD, 1, osl],
                                start=False, stop=True,
                            )
                            nc.vector.tensor_copy(
                                o_sb[:, io % 2, osl], pso
                            )
                        if finegrain:
                            dma_engines[io4].dma_start(
                                out_r[:, io : io + 1, :],
                                o_sb[:, io % 2 : io % 2 + 1, :],
                            )
                            if io % 2 == 1:
                                opair.pop(io // 2)
                        elif io % 2 == 1:
                            nc.sync.dma_start(
                                out_r[:, io - 1 : io + 1, :],
                                opair.pop(io // 2),
                            )

                    units.append(unit)
                return units

            def v_units(ic):
                units = []
                for io in range(ic * 4, ic * 4 + 4):
                    def unit(io=io):
                        ps = proj_p.tile([P, 512], F32, tag="proj")
                        psv = ps[:, :DV]
                        c0 = (io % 4) * P
                        for kc in range(KC):
                            nc.tensor.matmul(
                                psv,
                                lhsT=xt_sb[:, io // 4, kc, c0 : c0 + P],
                                rhs=wv_sb[:, kc, :],
                                start=(kc == 0),
                                stop=(kc == KC - 1),
                            )
                        nc.vector.tensor_copy(
                            v1[:, io, :, 0:HD],
                            psv.rearrange("p (h e) -> p h e", e=HD),
                        )
                    units.append(unit)
                return units

            # PE fillers for HAM: bare weight loads stream the array for
            # ~53ns each with NO psum write, so they never churn a tile
            # ring or create WAR stalls — pure duty-cycle padding.
            def ldw_fill(n=4):
                for _ in range(n):
                    nc.tensor.ldweights(warm_src[:, 0:P])

            # dummy matmuls keep PE activity above HAM's re-throttle window
            # through the sparse tail (they write scratch psum, never read)
            def dummy64():
                d = sc_p.tile([P, 2, 512], F32, tag="sc", name="dm64")
                nc.tensor.matmul(
                    d[0:65, 0, :], lhsT=warm_src[0:HD, 0:65],
                    rhs=warm_src[0:HD, 0:512], start=True, stop=True,
                )
                return d

            def dummy128():
                d = proj_p.tile([P, 512], F32, tag="proj", name="dm128")
                nc.tensor.matmul(
                    d, lhsT=warm_src[:, 0:P],
                    rhs=warm_src[:, 0:512], start=True, stop=True,
                )
                return d

            def run_phase(ic, work128, work64=(), keep_warm=False,
                          h2_delay=False, shift=0):
                # Emit scores for chunk ic in jb-slots; after each slot emit a
                # proportional share of 64-mode fillers (oproj) and 128-mode
                # work (V proj or AV of the larger chunk). shift=1 delays the
                # 128-mode stream by one slot so AV consumers trail the live
                # exp stream by a full slot of margin.
                carry[ic] = ([], [])
                n_j = 4 * ic + 4
                work64 = list(work64)
                work128 = list(work128)
                nslots = n_j // 2
                d64 = d128 = 0
                pend_h2 = None
                for s, jb in enumerate(range(0, n_j, 2)):
                    sc_group_pair(ic, jb)
                    sc_group_pair(ic, jb + 1)
                    if h2_delay:
                        if pend_h2 is not None:
                            sc_group_h2(ic, pend_h2)
                        pend_h2 = jb
                    else:
                        sc_group_h2(ic, jb)
                    w = len(work64) * (s + 1) // nslots
                    while d64 < w:
                        work64[d64]()
                        d64 += 1
                    if keep_warm:
                        dummy64()
                    w = len(work128) * max(0, s + 1 - shift) // nslots
                    while d128 < w:
                        work128[d128]()
                        d128 += 1
                    if keep_warm:
                        dummy128()
                if pend_h2 is not None:
                    sc_group_h2(ic, pend_h2)
                while d128 < len(work128):
                    work128[d128]()
                    d128 += 1

            # ---------------- main schedule ----------------
            # K(0) + Q(3) first so scores(3) -- and the ACT exp stream --
            # start as early as possible; the remaining K slices drip in as
            # 128-mode fillers just ahead of the key chunks that need them.
            # V/Q projections of the small chunks are pushed late to densify
            # the back half of the kernel (keeps HAM un-throttled).
            # dummy bridge BEFORE the first projection: the critical loads
            # land ~16-19us and the in-order PE queue would otherwise sit
            # idle from ~10us, resetting HAM's duty ramp — these fill the
            # window so k=8/8 arrives ~20us earlier. (Ring-safe: no proj
            # psum has readers yet.)
            for _ in range(24):
                dummy128()
            proj_slice(0, 0)   # K-pair(0): pair scores jc 0-3
            # bulk loads, gated on the first projections' evacuations so
            # they don't steal DMA bandwidth from the critical early loads
            gate_s = zr_p.tile([1, 8], BF16, tag="gate", name="gate_s")
            nc.scalar.activation(gate_s, klp[0:1, 0:8], AF.Copy)
            nc.scalar.dma_start(wv_sb, wv_d[:])
            nc.scalar.dma_start(wos_sb, wos_d[:])
            proj_slice(3, 2)   # Q-pair(3)
            gate_g = zr_p.tile([1, 8], BF16, tag="gate", name="gate_g")
            nc.gpsimd.tensor_tensor(
                gate_g, qTp[0:1, 1536:1544], qTp[0:1, 1536:1544], MUL
            )
            nc.gpsimd.dma_start(xt_sb[:, 2], xt_d[:, 2])
            nc.gpsimd.dma_start(mask_sb, mask_d[:])
            kdrip = [lambda: proj_slice(0, 1), lambda: proj_slice(3, 1)]
            for ic in (1, 2):
                kdrip.append(lambda ic=ic: proj_slice(ic, 0))
                kdrip.append(lambda ic=ic: proj_slice(ic, 1))
            kdrip.append(lambda: proj_slice(3, 0))
            run_phase(3, kdrip + [lambda: proj_slice(2, 2)]
                      + v_units(3) + v_units(2) + v_units(1) + v_units(0),
                      h2_delay=True)
            run_phase(2, av_stream(3) + [lambda: proj_slice(1, 2)], shift=1)
            run_phase(1, av_stream(2) + [lambda: proj_slice(0, 2)],
                      oproj_units(3), keep_warm=True, shift=1)
            run_phase(0, av_stream(1), oproj_units(2), keep_warm=True,
                      shift=1)
            # tail: oproj(1) units interleaved with av(0) + z-chains so the
            # PE stays dense while the (fast, DMA-free) z-chains resolve;
            # then the mults and chunk-0 output projections.
            av0 = av_stream(0)
            o1 = oproj_units(1)
            tail = [av0[0], av0[1], av0[2], o1[0], av0[3], av0[4], dummy128,
                    av0[5], av0[6], o1[1], av0[7], av0[12], dummy128,
                    av0[8], dummy128, av0[9], av0[10], o1[2], av0[11],
                    av0[13], o1[3], av0[14]]
            for item in tail:
                item()
            d64 = None
            for u in oproj_units(0, finegrain=True):
                u()
                d64 = dummy64()
            drain = zr_p.tile([HD, 512], F32, tag="ddr", name="ddrain")
            if d64 is not None:
                nc.vector.tensor_copy(drain, d64[0:HD, 0, :])

    _split_excess_waits(nc)
    return nc


_NC = None


def _get_nc():
    global _NC
    if _NC is None:
        _NC = _build_module()
    return _NC


def _make_mask():
    p = np.arange(P)[:, None]
    f = np.arange(512)[None, :]
    m = np.empty((P, 4, 2, 512), np.float32)
    for k in range(4):
        m[:, k, 0, :] = (p <= f - P * k).astype(np.float32)
        m[:, k, 1, :] = m[:, k, 0, :]
    return m.astype(NPBF16)


def _build_in_maps(x, wq, bq, wk, bk, wv, bv, wo):
    scale = 1.0 / np.sqrt(HD)
    mask = _make_mask()
    in_maps = []
    for core in range(8):
        b = core // 4
        h0 = (core % 4) * HPC

        # pair-packed slices: [K_h0|K_h1], [K_h2|Q_h2], [Q_h0|Q_h1]
        wqk = np.empty((D, DQK), np.float32)
        bqk = np.empty((P, HPC), np.float32)
        cs = [slice((h0 + i) * HD, (h0 + i + 1) * HD) for i in range(HPC)]
        wqk[:, 0:HD] = wk[:, cs[0]]
        wqk[:, HD:P] = wk[:, cs[1]]
        wqk[:, P : P + HD] = wk[:, cs[2]]
        wqk[:, P + HD : 2 * P] = wq[:, cs[2]] * scale
        wqk[:, 2 * P : 2 * P + HD] = wq[:, cs[0]] * scale
        wqk[:, 2 * P + HD : 3 * P] = wq[:, cs[1]] * scale
        bqk[0:HD, 0] = bk[cs[0]]
        bqk[HD:P, 0] = bk[cs[1]]
        bqk[0:HD, 1] = bk[cs[2]]
        bqk[HD:P, 1] = bq[cs[2]] * scale
        bqk[0:HD, 2] = bq[cs[0]] * scale
        bqk[HD:P, 2] = bq[cs[1]] * scale

        vcols = slice(h0 * HD, (h0 + HPC) * HD)
        # wos: [P, 2, D] with h0/h1 stacked in slot 0 (K=128 oproj matmul)
        # and h2 in rows 0:64 of slot 1
        w_all = wo[vcols, :].reshape(HPC, HD, D)
        wos = np.zeros((P, 2, D), np.float32)
        wos[0:HD, 0] = w_all[0]
        wos[HD:P, 0] = w_all[1]
        wos[0:HD, 1] = w_all[2]

        # pre-rearranged for one-descriptor-per-partition DMA loads
        xt = (
            x[b].T.reshape(KC, P, IC, 512).transpose(1, 2, 0, 3)
        )  # [P, IC, KC, 512]
        wqk_r = wqk.reshape(KC, P, HPC, P).transpose(1, 2, 0, 3)
        wv_r = wv[:, vcols].reshape(KC, P, DV).transpose(1, 0, 2)

        in_maps.append(
            {
                "xt": np.ascontiguousarray(xt).astype(NPBF16),
                "wqk": np.ascontiguousarray(wqk_r).astype(NPBF16),
                "bqk": bqk.astype(np.float32),
                "wv": np.ascontiguousarray(wv_r).astype(NPBF16),
                "wos": np.ascontiguousarray(wos).astype(NPBF16),
                "mask": mask,
            }
        )
    return in_maps


def kernel(x, wq, bq, wk, bk, wv, bv, wo, bo):
    x = np.asarray(x, np.float32)
    wq = np.asarray(wq, np.float32)
    bq = np.asarray(bq, np.float32)
    wk = np.asarray(wk, np.float32)
    bk = np.asarray(bk, np.float32)
    wv = np.asarray(wv, np.float32)
    bv = np.asarray(bv, np.float32)
    wo = np.asarray(wo, np.float32)
    bo = np.asarray(bo, np.float32)

    in_maps = _build_in_maps(x, wq, bq, wk, bk, wv, bv, wo)
    res = run_bass_kernel_spmd(_get_nc(), in_maps, core_ids=list(range(8)))
    out = np.zeros((B, S, D), np.float32)
    for core in range(8):
        out[core // 4] += np.asarray(res.results[core]["out"], np.float32)
    out += bo + bv @ wo
    return out



# revision 58
# speedup vs baseline: 1.0665x; 1.0665x over previous
# Multi-head causal self-attention (B=2, S=2048, D=768, H=12) on 8 NeuronCores.
#
# Sharding: (batch, head-group) across cores. Core c handles batch c//4 and
# heads 3*(c%4) .. 3*(c%4)+2. Each core computes its heads' Q/K/V projections
# (column-sharded), the causal attention for those heads, and a row-sharded
# partial of the output projection. Host sums the 4 partials per batch + bo.
#
# Engine plan (v5):
#  - PE batches work by tile-size mode so the array never mode-switch-drains
#    mid-stream: (128,128) for QK/V projections and AV (M=65, ones column
#    accumulates the softmax denominator); (64,128) for scores (heads 0,1
#    pair-stacked on partitions and issued to row tiles T0/T8 which run
#    concurrently; head 2 on T0) and the K=64 output projection.
#  - Attention chunks are processed in DESCENDING size order (ic3..ic0) with
#    all QK projections hoisted to the front: the long exp streams start
#    early and the kernel tail is the smallest chunk.
#  - ACT runs exp and the z-chain. 1/Z = exp(-ln Z), entirely on-engine
#    (no DMA round trips): ACT Ln reads the Z row straight from PSUM, a
#    K=1 fp32 matmul broadcasts ln Z into the AV tile's unused partitions
#    64:128, ACT Exp(scale=-1) evacuates 1/Z to SBUF, DVE multiplies.
#    (ACT Reciprocal is blocked in bass for accuracy; Ln+Exp is accurate.)
#  - Initial loads are batched into few triggers spread over 4 engine
#    queues so the first projection data lands ~6us in; PE ramps HAM to
#    full duty on real work instead of idling on DMA dispatch.
#  - GPSIMD: causal-mask multiplies. Output DMA'd in bf16; host sums in fp32.

import sys

import ml_dtypes
import numpy as np

sys.path.insert(0, "/opt/trn_rl_repo")

import concourse.bass as bass  # noqa: E402
import concourse.mybir as mybir  # noqa: E402
import concourse.tile as tile  # noqa: E402
from concourse.bass import ts  # noqa: E402
from concourse.bass_utils import run_bass_kernel_spmd  # noqa: E402

F32 = mybir.dt.float32
BF16 = mybir.dt.bfloat16
AF = mybir.ActivationFunctionType
MUL = mybir.AluOpType.mult
ADD = mybir.AluOpType.add
NPBF16 = ml_dtypes.bfloat16

B, S, D, H, HD = 2, 2048, 768, 12, 64
HPC = 3               # heads per core
DQK = 2 * HPC * HD    # 384
DV = HPC * HD         # 192
P = 128
IC = S // 512         # 4 query chunks of 512
KC = D // P           # 6 contraction chunks
NIO = S // P          # 16 token chunks of 128


def _split_excess_waits(nc, max_waits=1):
    # walrus in this env rejects instructions carrying more than ~1-2
    # sync-waits. Move excess waits onto preceding same-engine nops.
    n_split = 0
    for func in nc.m.functions:
        for blk in func.blocks:
            insts = blk.instructions
            out = []
            changed = False
            for inst in insts:
                si = inst.sync_info
                waits = list(si.on_wait) if si and si.on_wait else []
                if len(waits) > max_waits:
                    changed = True
                    for j, w in enumerate(waits[:-max_waits]):
                        out.append(
                            mybir.InstNoOp(
                                name=f"{inst.name}-wsplit{j}",
                                engine=inst.engine,
                                ins=[],
                                outs=[],
                                sync_info=mybir.SyncInfo(
                                    on_wait=[w], on_update=[]
                                ),
                            )
                        )
                        n_split += 1
                    inst.sync_info = mybir.SyncInfo(
                        on_wait=waits[-max_waits:],
                        on_update=list(si.on_update) if si.on_update else [],
                    )
                out.append(inst)
            if changed:
                blk.instructions = out
    return n_split


def _build_module():
    # All weight/activation DRAM tensors are host-pre-rearranged so that
    # every SBUF load is one descriptor per partition (contiguous src and
    # dst): descriptor generation is ~10ns/descriptor, so 128 fat
    # descriptors start flowing ~7us earlier than 768 thin ones.
    nc = bass.Bass()
    xt_d = nc.dram_tensor("xt", [P, IC, KC, 512], BF16, kind="ExternalInput")
    wqk_d = nc.dram_tensor("wqk", [P, HPC, KC, P], BF16, kind="ExternalInput")
    bqk_d = nc.dram_tensor("bqk", [P, HPC], F32, kind="ExternalInput")
    wv_d = nc.dram_tensor("wv", [P, KC, DV], BF16, kind="ExternalInput")
    wos_d = nc.dram_tensor("wos", [P, 2, D], BF16, kind="ExternalInput")
    mask_d = nc.dram_tensor("mask", [P, 4, 2, 512], BF16, kind="ExternalInput")
    out_d = nc.dram_tensor("out", [S, D], BF16, kind="ExternalOutput")
    scratch_d = nc.dram_tensor("scratch", [HD + 1, 512], F32)
    gate_d = nc.dram_tensor("gatescr", [1, 8], BF16)

    with tile.TileContext(nc) as tc:
        with (
            tc.tile_pool(name="const", bufs=1) as cp,
            tc.tile_pool(name="exp", bufs=40) as exp_p,
            tc.tile_pool(name="zr", bufs=2) as zr_p,
            tc.tile_pool(name="outp", bufs=2) as op,
            tc.tile_pool(name="proj", bufs=2, space="PSUM") as proj_p,
            tc.tile_pool(name="scps", bufs=2, space="PSUM") as sc_p,
            tc.tile_pool(name="avps", bufs=2, space="PSUM") as av_p,
        ):
            # ---- PE warm-up source via DVE memset (gpsimd starts slowly) ----
            warm_src = cp.tile([P, 520], BF16)
            nc.vector.memset(warm_src, 1.0)

            # ---- resident SBUF tensors ----
            # The 16 hw DMA engines are a shared ~250GB/s pool: concurrent
            # transfers steal bandwidth from each other, so the critical
            # first-projection loads (xt chunk 0 + wqk) are issued alone;
            # the bulk (mask/wv/wos, xt chunks 1+2) is gated behind tiny
            # compute ops that only unblock once the first projections are
            # evacuating (~13us), keeping the early window clean.
            wqk_sb = cp.tile([P, HPC, KC, P], BF16)
            xt_sb = cp.tile([P, IC, KC, 512], BF16)
            bqk_sb = cp.tile([P, HPC], F32)
            wv_sb = cp.tile([P, KC, DV], BF16)
            mask_sb = cp.tile([P, 4, 2, 512], BF16)
            wos_sb = cp.tile([P, 2, D], BF16)

            # The DMA engines round-robin among ALL active transfers
            # (~250GB/s shared, data flow starts ~8.5us in), so issue the
            # early-needed set together and gate only the genuinely
            # late-needed bulk (xt chunk 2, mask, wv, wos) behind compute.
            nc.sync.dma_start(wqk_sb[:, 0], wqk_d[:, 0])
            nc.sync.dma_start(xt_sb[:, 0], xt_d[:, 0])
            nc.sync.dma_start(wqk_sb[:, 1:3], wqk_d[:, 1:3])
            nc.sync.dma_start(xt_sb[:, 3], xt_d[:, 3])
            nc.sync.dma_start(bqk_sb, bqk_d[:])
            nc.sync.dma_start(xt_sb[:, 1], xt_d[:, 1])

            # V with ones columns HD:P: every AV psum row 64:128 then
            # accumulates the softmax denominator Z, pre-broadcast, and the
            # AV matmul gets a full 128-wide stationary operand. Cols 0:HD
            # are overwritten by the V-projection evacuations.
            v1 = cp.tile([P, NIO, HPC, P], BF16)
            nc.vector.memset(v1[:, :, :, HD:P], 1.0)

            # pair-stacked Q^T/K^T for heads 0,1; head 2's K/Q duplicated
            # into both partition halves so its scores pair on T0/T8 too
            qTp = cp.tile([P, S], BF16)
            klp = cp.tile([P, S], BF16)
            qT2d = cp.tile([P, S], BF16)
            kl2d = cp.tile([P, S], BF16)
            # ctx for h0/h1 stacked on partitions: the output projection
            # contracts both heads in one K=128 matmul; h2 separate (K=64)
            ctxT01 = cp.tile([P, S], BF16)
            ctxT2 = cp.tile([HD, S], BF16)

            # ---- PE warm-up: (128,128)-mode matmuls ----
            # Just enough to bridge until the first projection's DMA lands
            # (~6us); the projections themselves sustain the HAM ramp.
            warm_ps = av_p.tile([P, 512], F32, tag="av", name="warm")
            for w in range(5):
                nc.tensor.matmul(
                    warm_ps[0 : HD + 1, :],
                    lhsT=warm_src[:, 0:65],
                    rhs=warm_src[:, 0:512],
                    start=True,
                    stop=(w == 4),
                )
            warm_sb = zr_p.tile([HD + 1, 512], F32, tag="warm", name="warmsb")
            nc.vector.tensor_copy(warm_sb, warm_ps[0 : HD + 1, :])
            nc.sync.dma_start(scratch_d[:], warm_sb)

            carry = {}   # ic -> (pair ex tiles, h2 ex tiles)

            def proj_slice(ic, sl):
                # wqk slices: 0 -> [K_h0|K_h1], 1 -> [K_h2|Q_h2],
                #             2 -> [Q_h0|Q_h1]
                isl = ts(ic, 512)
                ps = proj_p.tile([P, 512], F32, tag="proj")
                for kc in range(KC):
                    nc.tensor.matmul(
                        ps,
                        lhsT=wqk_sb[:, sl, kc, :],
                        rhs=xt_sb[:, ic, kc, :],
                        start=(kc == 0),
                        stop=(kc == KC - 1),
                    )
                if sl == 0:
                    nc.vector.tensor_scalar(
                        klp[:, isl], ps, bqk_sb[:, 0:1], None, ADD,
                    )
                elif sl == 1:
                    # duplicate K_h2/Q_h2 into both partition halves
                    nc.vector.tensor_scalar(
                        kl2d[0:HD, isl], ps[0:HD, :], bqk_sb[0:HD, 1:2],
                        None, ADD,
                    )
                    nc.vector.tensor_scalar(
                        kl2d[HD:P, isl], ps[0:HD, :], bqk_sb[0:HD, 1:2],
                        None, ADD,
                    )
                    nc.vector.tensor_scalar(
                        qT2d[0:HD, isl], ps[HD:P, :], bqk_sb[HD:P, 1:2],
                        None, ADD,
                    )
                    nc.vector.tensor_scalar(
                        qT2d[HD:P, isl], ps[HD:P, :], bqk_sb[HD:P, 1:2],
                        None, ADD,
                    )
                else:
                    nc.vector.tensor_scalar(
                        qTp[:, isl], ps, bqk_sb[:, 2:3], None, ADD,
                    )

            def trim_of(jc, ic):
                koff = jc - 4 * ic
                return P * koff if koff > 0 else 0

            def sc_group_pair(ic, jc):
                t = trim_of(jc, ic)
                koff = jc - 4 * ic
                sc = sc_p.tile([P, 2, 512], F32, tag="sc", name=f"sp{ic}_{jc}")
                for h in range(2):
                    hsl = ts(h, HD)
                    nc.tensor.matmul(
                        sc[:, h, t:],
                        lhsT=klp[hsl, ts(jc, P)],
                        rhs=qTp[hsl, ic * 512 + t : (ic + 1) * 512],
                        start=True,
                        stop=True,
                    )
                ex = exp_p.tile([P, 2, 512], BF16, tag="ex", name=f"xp{ic}_{jc}")
                nc.scalar.activation(ex[:, :, t:], sc[:, :, t:], AF.Exp)
                if koff >= 0:
                    nc.gpsimd.tensor_tensor(
                        ex[:, :, t:], ex[:, :, t:],
                        mask_sb[:, koff, :, t:], MUL,
                    )
                carry[ic][0].append(ex)

            def sc_group_h2(ic, jb):
                # the two key-blocks go to T0/T8 concurrently via the
                # duplicated partition halves of kl2d/qT2d
                sc = sc_p.tile([P, 2, 512], F32, tag="sc", name=f"s2_{ic}_{jb}")
                for k in range(2):
                    jc = jb + k
                    t = trim_of(jc, ic)
                    hs = slice(k * HD, (k + 1) * HD)
                    nc.tensor.matmul(
                        sc[:, k, t:],
                        lhsT=kl2d[hs, ts(jc, P)],
                        rhs=qT2d[hs, ic * 512 + t : (ic + 1) * 512],
                        start=True,
                        stop=True,
                    )
                ex = exp_p.tile([P, 2, 512], BF16, tag="ex", name=f"x2_{ic}_{jb}")
                koff = jb - 4 * ic
                if koff >= 0 and trim_of(jb + 1, ic) > 0:
                    for k in range(2):
                        t = trim_of(jb + k, ic)
                        nc.scalar.activation(ex[:, k, t:], sc[:, k, t:], AF.Exp)
                        nc.gpsimd.tensor_tensor(
                            ex[:, k, t:], ex[:, k, t:],
                            mask_sb[:, koff + k, 0, t:], MUL,
                        )
                else:
                    nc.scalar.activation(ex, sc, AF.Exp)
                    if koff >= 0:
                        nc.gpsimd.tensor_tensor(
                            ex, ex, mask_sb[:, koff : koff + 2, 0, :], MUL,
                        )
                carry[ic][1].append(ex)

            # z chains, fully on ACT: the AV psum rows HD:P already hold Z
            # pre-broadcast (ones columns of v1), so 1/Z = exp(-ln Z) is
            # two partition-parallel ACT table ops, no DMA, no PE.
            # (custom-DVE reciprocal_approx_fast would be cheaper still but
            # the axon compile path can't emit custom DVE ISA ops.)
            zbs = {}    # (ic, h) -> zb tile

            def make_zchain_a(ic, h, state):
                def go():
                    lnb = zr_p.tile([HD, 512], F32, tag="lnz",
                                    name=f"ln{ic}{h}")
                    nc.scalar.activation(lnb, state[h][HD:P, :], AF.Ln)
                    zbs[(ic, h, "ln")] = lnb
                return go

            def make_zchain_b(ic, h, state):
                def go():
                    lnb = zbs.pop((ic, h, "ln"))
                    zb = zr_p.tile([HD, 512], F32, tag="zb", name=f"zb{ic}{h}")
                    nc.scalar.activation(zb, lnb, AF.Exp, scale=-1.0)
                    zbs[(ic, h)] = zb
                return go

            def make_mult(ic, h, state, parts=1):
                def go():
                    avt = state.pop(h)
                    zb = zbs.pop((ic, h))
                    w = 512 // parts
                    for c in range(parts):
                        cs = slice(c * w, (c + 1) * w)
                        osl = slice(ic * 512 + c * w, ic * 512 + (c + 1) * w)
                        if h == 0:
                            out = ctxT01[0:HD, osl]
                        elif h == 1:
                            out = ctxT01[HD:P, osl]
                        else:
                            out = ctxT2[:, osl]
                        nc.vector.tensor_tensor(
                            out, avt[0:HD, cs], zb[:, cs], MUL,
                        )
                return go

            def av_stream(ic):
                # mm batches + fin, with each head's mult deferred one head
                n_j = 4 * ic + 4
                state = {}
                work = []

                def mk_mm(h, j0, j1):
                    def go():
                        if h not in state:
                            if ic == 0 and h == 2:
                                t_ = sc_p.tile([P, 2, 512], F32, tag="sc",
                                               name=f"av{ic}{h}")
                                state[h] = t_[:, 0, :]
                            else:
                                state[h] = av_p.tile(
                                    [P, 512], F32, tag="av", name=f"av{ic}{h}"
                                )
                        avt = state[h]
                        exs, exs2 = carry[ic]
                        for jc in range(j0, j1):
                            t = trim_of(jc, ic)
                            exap = (exs[jc][:, h, t:] if h < 2
                                    else exs2[jc // 2][:, jc % 2, t:])
                            nc.tensor.matmul(
                                avt[:, t:],
                                lhsT=v1[:, jc, h, :],
                                rhs=exap,
                                start=(jc == 0),
                                stop=(jc == n_j - 1),
                            )
                    return go

                if ic == 0:
                    # tail chunk: h2's AV accumulates in a free sc-pool bank,
                    # so all three AVs and z-chains overlap before any mult.
                    # Mults split 2x256 so oproj(0) unblocks per half-chunk.
                    for h in range(HPC):
                        for j0 in range(0, n_j, 2):
                            work.append(mk_mm(h, j0, min(j0 + 2, n_j)))
                        work.append(make_zchain_a(ic, h, state))
                        work.append(make_zchain_b(ic, h, state))
                    for h in range(HPC):
                        work.append(make_mult(ic, h, state, parts=2))
                    return work
                for h in range(HPC):
                    # batches of 2 key-chunks with a dummy before every
                    # batch: in the ACT-bound phases the PE inevitably
                    # waits on the live exp stream, and the dummies turn
                    # that wait into HAM-visible activity (idle epochs
                    # halve the duty cycle for 10-30us).
                    mms = [mk_mm(h, j0, min(j0 + 2, n_j))
                           for j0 in range(0, n_j, 2)]
                    items = []
                    for i, m in enumerate(mms):
                        if i >= len(mms) - 2:
                            items.append(dummy128)
                        items.append(m)
                    if h >= 1:
                        items.insert(1, make_zchain_b(ic, h - 1, state))
                        items.insert(2, make_mult(ic, h - 1, state))
                    items.append(make_zchain_a(ic, h, state))
                    work.extend(items)
                work.append(make_zchain_b(ic, HPC - 1, state))
                work.append(make_mult(ic, HPC - 1, state))
                return work

            out_r = out_d.rearrange("(io p) d -> p io d", p=P)
            opair = {}

            def oproj_units(ic, finegrain=False):
                # units in io pairs sharing one o_sb tile and one output DMA;
                # finegrain (tail chunk): one DMA per io on rotating queues
                # so the last write isn't one big serialized transfer.
                units = []
                dma_engines = (nc.sync, nc.gpsimd, nc.scalar, nc.sync)
                for io4 in range(4):
                    io = ic * 4 + io4

                    def unit(io=io, io4=io4):
                        if io % 2 == 0:
                            opair[io // 2] = op.tile(
                                [P, 2, D], BF16, tag="osb", name=f"ou{io}"
                            )
                        o_sb = opair[io // 2]
                        for ot, ow in ((0, 512), (1, 256)):
                            ps = proj_p.tile([P, 512], F32, tag="proj")
                            pso = ps[:, :ow]
                            osl = slice(ot * 512, ot * 512 + ow)
                            nc.tensor.matmul(
                                pso, lhsT=ctxT01[:, ts(io, P)],
                                rhs=wos_sb[:, 0, osl],
                                start=True, stop=False,
                            )
                            nc.tensor.matmul(
                                pso, lhsT=ctxT2[:, ts(io, P)],
                                rhs=wos_sb[0:HD, 1, osl],
                                start=False, stop=True,
                            )
                            nc.vector.tensor_copy(
                                o_sb[:, io % 2, osl], pso
                            )
                        if finegrain:
                            dma_engines[io4].dma_start(
                                out_r[:, io : io + 1, :],
                                o_sb[:, io % 2 : io % 2 + 1, :],
                            )
                            if io % 2 == 1:
                                opair.pop(io // 2)
                        elif io % 2 == 1:
                            nc.sync.dma_start(
                                out_r[:, io - 1 : io + 1, :],
                                opair.pop(io // 2),
                            )

                    units.append(unit)
                return units

            def v_units(ic):
                units = []
                for io in range(ic * 4, ic * 4 + 4):
                    def unit(io=io):
                        ps = proj_p.tile([P, 512], F32, tag="proj")
                        psv = ps[:, :DV]
                        c0 = (io % 4) * P
                        for kc in range(KC):
                            nc.tensor.matmul(
                                psv,
                                lhsT=xt_sb[:, io // 4, kc, c0 : c0 + P],
                                rhs=wv_sb[:, kc, :],
                                start=(kc == 0),
                                stop=(kc == KC - 1),
                            )
                        nc.vector.tensor_copy(
                            v1[:, io, :, 0:HD],
                            psv.rearrange("p (h e) -> p h e", e=HD),
                        )
                    units.append(unit)
                return units

            # PE fillers for HAM: bare weight loads stream the array for
            # ~53ns each with NO psum write, so they never churn a tile
            # ring or create WAR stalls — pure duty-cycle padding.
            def ldw_fill(n=4):
                for _ in range(n):
                    nc.tensor.ldweights(warm_src[:, 0:P])

            # dummy matmuls keep PE activity above HAM's re-throttle window
            # through the sparse tail (they write scratch psum, never read)
            def dummy64():
                d = sc_p.tile([P, 2, 512], F32, tag="sc", name="dm64")
                nc.tensor.matmul(
                    d[0:65, 0, :], lhsT=warm_src[0:HD, 0:65],
                    rhs=warm_src[0:HD, 0:512], start=True, stop=True,
                )
                return d

            def dummy128():
                d = proj_p.tile([P, 512], F32, tag="proj", name="dm128")
                nc.tensor.matmul(
                    d, lhsT=warm_src[:, 0:P],
                    rhs=warm_src[:, 0:512], start=True, stop=True,
                )
                return d

            def run_phase(ic, work128, work64=(), keep_warm=False,
                          h2_delay=False, shift=0):
                # Emit scores for chunk ic in jb-slots; after each slot emit a
                # proportional share of 64-mode fillers (oproj) and 128-mode
                # work (V proj or AV of the larger chunk). shift=1 delays the
                # 128-mode stream by one slot so AV consumers trail the live
                # exp stream by a full slot of margin.
                carry[ic] = ([], [])
                n_j = 4 * ic + 4
                work64 = list(work64)
                work128 = list(work128)
                nslots = n_j // 2
                d64 = d128 = 0
                pend_h2 = None
                for s, jb in enumerate(range(0, n_j, 2)):
                    sc_group_pair(ic, jb)
                    sc_group_pair(ic, jb + 1)
                    if h2_delay:
                        if pend_h2 is not None:
                            sc_group_h2(ic, pend_h2)
                        pend_h2 = jb
                    else:
                        sc_group_h2(ic, jb)
                    w = len(work64) * (s + 1) // nslots
                    while d64 < w:
                        work64[d64]()
                        d64 += 1
                    if keep_warm:
                        dummy64()
                    w = len(work128) * max(0, s + 1 - shift) // nslots
                    while d128 < w:
                        work128[d128]()
                        d128 += 1
                    if keep_warm:
                        dummy128()
                if pend_h2 is not None:
                    sc_group_h2(ic, pend_h2)
                while d128 < len(work128):
                    work128[d128]()
                    d128 += 1

            # ---------------- main schedule ----------------
            # K(0) + Q(3) first so scores(3) -- and the ACT exp stream --
            # start as early as possible; the remaining K slices drip in as
            # 128-mode fillers just ahead of the key chunks that need them.
            # V/Q projections of the small chunks are pushed late to densify
            # the back half of the kernel (keeps HAM un-throttled).
            proj_slice(0, 0)   # K-pair(0): pair scores jc 0-3
            # bulk loads, gated on the first projections' evacuations so
            # they don't steal DMA bandwidth from the critical early loads
            gate_s = zr_p.tile([1, 8], BF16, tag="gate", name="gate_s")
            nc.scalar.activation(gate_s, klp[0:1, 0:8], AF.Copy)
            nc.scalar.dma_start(wv_sb, wv_d[:])
            nc.scalar.dma_start(wos_sb, wos_d[:])
            proj_slice(3, 2)   # Q-pair(3)
            gate_g = zr_p.tile([1, 8], BF16, tag="gate", name="gate_g")
            nc.gpsimd.tensor_tensor(
                gate_g, qTp[0:1, 1536:1544], qTp[0:1, 1536:1544], MUL
            )
            nc.gpsimd.dma_start(xt_sb[:, 2], xt_d[:, 2])
            nc.gpsimd.dma_start(mask_sb, mask_d[:])
            kdrip = [lambda: proj_slice(0, 1), lambda: proj_slice(3, 1)]
            for ic in (1, 2):
                kdrip.append(lambda ic=ic: proj_slice(ic, 0))
                kdrip.append(lambda ic=ic: proj_slice(ic, 1))
            kdrip.append(lambda: proj_slice(3, 0))
            run_phase(3, kdrip + [lambda: proj_slice(2, 2)]
                      + v_units(3) + v_units(2) + v_units(1) + v_units(0),
                      h2_delay=True)
            run_phase(2, av_stream(3) + [lambda: proj_slice(1, 2)], shift=2)
            run_phase(1, av_stream(2) + [lambda: proj_slice(0, 2)],
                      oproj_units(3), keep_warm=True, shift=2)
            run_phase(0, av_stream(1), oproj_units(2), keep_warm=True,
                      shift=2)
            # tail: oproj(1) units interleaved with av(0) + z-chains so the
            # PE stays dense while the (fast, DMA-free) z-chains resolve;
            # then the mults and chunk-0 output projections.
            av0 = av_stream(0)
            o1 = oproj_units(1)
            tail = [av0[0], av0[1], av0[2], o1[0], av0[3], av0[4], dummy128,
                    av0[5], av0[6], o1[1], av0[7], av0[12], dummy128,
                    av0[8], dummy128, av0[9], av0[10], o1[2], av0[11],
                    av0[13], o1[3], av0[14]]
            for item in tail:
                item()
            d64 = None
            for u in oproj_units(0, finegrain=True):
                u()
                d64 = dummy64()
            drain = zr_p.tile([HD, 512], F32, tag="ddr", name="ddrain")
            if d64 is not None:
                nc.vector.tensor_copy(drain, d64[0:HD, 0, :])

    _split_excess_waits(nc)
    return nc


_NC = None


def _get_nc():
    global _NC
    if _NC is None:
        _NC = _build_module()
    return _NC


def _make_mask():
    p = np.arange(P)[:, None]
    f = np.arange(512)[None, :]
    m = np.empty((P, 4, 2, 512), np.float32)
    for k in range(4):
        m[:, k, 0, :] = (p <= f - P * k).astype(np.float32)
        m[:, k, 1, :] = m[:, k, 0, :]
    return m.astype(NPBF16)


def _build_in_maps(x, wq, bq, wk, bk, wv, bv, wo):
    scale = 1.0 / np.sqrt(HD)
    mask = _make_mask()
    in_maps = []
    for core in range(8):
        b = core // 4
        h0 = (core % 4) * HPC

        # pair-packed slices: [K_h0|K_h1], [K_h2|Q_h2], [Q_h0|Q_h1]
        wqk = np.empty((D, DQK), np.float32)
        bqk = np.empty((P, HPC), np.float32)
        cs = [slice((h0 + i) * HD, (h0 + i + 1) * HD) for i in range(HPC)]
        wqk[:, 0:HD] = wk[:, cs[0]]
        wqk[:, HD:P] = wk[:, cs[1]]
        wqk[:, P : P + HD] = wk[:, cs[2]]
        wqk[:, P + HD : 2 * P] = wq[:, cs[2]] * scale
        wqk[:, 2 * P : 2 * P + HD] = wq[:, cs[0]] * scale
        wqk[:, 2 * P + HD : 3 * P] = wq[:, cs[1]] * scale
        bqk[0:HD, 0] = bk[cs[0]]
        bqk[HD:P, 0] = bk[cs[1]]
        bqk[0:HD, 1] = bk[cs[2]]
        bqk[HD:P, 1] = bq[cs[2]] * scale
        bqk[0:HD, 2] = bq[cs[0]] * scale
        bqk[HD:P, 2] = bq[cs[1]] * scale

        vcols = slice(h0 * HD, (h0 + HPC) * HD)
        # wos: [P, 2, D] with h0/h1 stacked in slot 0 (K=128 oproj matmul)
        # and h2 in rows 0:64 of slot 1
        w_all = wo[vcols, :].reshape(HPC, HD, D)
        wos = np.zeros((P, 2, D), np.float32)
        wos[0:HD, 0] = w_all[0]
        wos[HD:P, 0] = w_all[1]
        wos[0:HD, 1] = w_all[2]

        # pre-rearranged for one-descriptor-per-partition DMA loads
        xt = (
            x[b].T.reshape(KC, P, IC, 512).transpose(1, 2, 0, 3)
        )  # [P, IC, KC, 512]
        wqk_r = wqk.reshape(KC, P, HPC, P).transpose(1, 2, 0, 3)
        wv_r = wv[:, vcols].reshape(KC, P, DV).transpose(1, 0, 2)

        in_maps.append(
            {
                "xt": np.ascontiguousarray(xt).astype(NPBF16),
                "wqk": np.ascontiguousarray(wqk_r).astype(NPBF16),
                "bqk": bqk.astype(np.float32),
                "wv": np.ascontiguousarray(wv_r).astype(NPBF16),
                "wos": np.ascontiguousarray(wos).astype(NPBF16),
                "mask": mask,
            }
        )
    return in_maps


def kernel(x, wq, bq, wk, bk, wv, bv, wo, bo):
    x = np.asarray(x, np.float32)
    wq = np.asarray(wq, np.float32)
    bq = np.asarray(bq, np.float32)
    wk = np.asarray(wk, np.float32)
    bk = np.asarray(bk, np.float32)
    wv = np.asarray(wv, np.float32)
    bv = np.asarray(bv, np.float32)
    wo = np.asarray(wo, np.float32)
    bo = np.asarray(bo, np.float32)

    in_maps = _build_in_maps(x, wq, bq, wk, bk, wv, bv, wo)
    res = run_bass_kernel_spmd(_get_nc(), in_maps, core_ids=list(range(8)))
    out = np.zeros((B, S, D), np.float32)
    for core in range(8):
        out[core // 4] += np.asarray(res.results[core]["out"], np.float32)
    out += bo + bv @ wo
    return out



# revision 59
# speedup vs baseline: 1.0791x; 1.0118x over previous
# Multi-head causal self-attention (B=2, S=2048, D=768, H=12) on 8 NeuronCores.
#
# Sharding: (batch, head-group) across cores. Core c handles batch c//4 and
# heads 3*(c%4) .. 3*(c%4)+2. Each core computes its heads' Q/K/V projections
# (column-sharded), the causal attention for those heads, and a row-sharded
# partial of the output projection. Host sums the 4 partials per batch + bo.
#
# Engine plan (v5):
#  - PE batches work by tile-size mode so the array never mode-switch-drains
#    mid-stream: (128,128) for QK/V projections and AV (M=65, ones column
#    accumulates the softmax denominator); (64,128) for scores (heads 0,1
#    pair-stacked on partitions and issued to row tiles T0/T8 which run
#    concurrently; head 2 on T0) and the K=64 output projection.
#  - Attention chunks are processed in DESCENDING size order (ic3..ic0) with
#    all QK projections hoisted to the front: the long exp streams start
#    early and the kernel tail is the smallest chunk.
#  - ACT runs exp and the z-chain. 1/Z = exp(-ln Z), entirely on-engine
#    (no DMA round trips): ACT Ln reads the Z row straight from PSUM, a
#    K=1 fp32 matmul broadcasts ln Z into the AV tile's unused partitions
#    64:128, ACT Exp(scale=-1) evacuates 1/Z to SBUF, DVE multiplies.
#    (ACT Reciprocal is blocked in bass for accuracy; Ln+Exp is accurate.)
#  - Initial loads are batched into few triggers spread over 4 engine
#    queues so the first projection data lands ~6us in; PE ramps HAM to
#    full duty on real work instead of idling on DMA dispatch.
#  - GPSIMD: causal-mask multiplies. Output DMA'd in bf16; host sums in fp32.

import sys

import ml_dtypes
import numpy as np

sys.path.insert(0, "/opt/trn_rl_repo")

import concourse.bass as bass  # noqa: E402
import concourse.mybir as mybir  # noqa: E402
import concourse.tile as tile  # noqa: E402
from concourse.bass import ts  # noqa: E402
from concourse.bass_utils import run_bass_kernel_spmd  # noqa: E402

F32 = mybir.dt.float32
BF16 = mybir.dt.bfloat16
AF = mybir.ActivationFunctionType
MUL = mybir.AluOpType.mult
ADD = mybir.AluOpType.add
NPBF16 = ml_dtypes.bfloat16

B, S, D, H, HD = 2, 2048, 768, 12, 64
HPC = 3               # heads per core
DQK = 2 * HPC * HD    # 384
DV = HPC * HD         # 192
P = 128
IC = S // 512         # 4 query chunks of 512
KC = D // P           # 6 contraction chunks
NIO = S // P          # 16 token chunks of 128


def _split_excess_waits(nc, max_waits=1):
    # walrus in this env rejects instructions carrying more than ~1-2
    # sync-waits. Move excess waits onto preceding same-engine nops.
    n_split = 0
    for func in nc.m.functions:
        for blk in func.blocks:
            insts = blk.instructions
            out = []
            changed = False
            for inst in insts:
                si = inst.sync_info
                waits = list(si.on_wait) if si and si.on_wait else []
                if len(waits) > max_waits:
                    changed = True
                    for j, w in enumerate(waits[:-max_waits]):
                        out.append(
                            mybir.InstNoOp(
                                name=f"{inst.name}-wsplit{j}",
                                engine=inst.engine,
                                ins=[],
                                outs=[],
                                sync_info=mybir.SyncInfo(
                                    on_wait=[w], on_update=[]
                                ),
                            )
                        )
                        n_split += 1
                    inst.sync_info = mybir.SyncInfo(
                        on_wait=waits[-max_waits:],
                        on_update=list(si.on_update) if si.on_update else [],
                    )
                out.append(inst)
            if changed:
                blk.instructions = out
    return n_split


def _build_module():
    # All weight/activation DRAM tensors are host-pre-rearranged so that
    # every SBUF load is one descriptor per partition (contiguous src and
    # dst): descriptor generation is ~10ns/descriptor, so 128 fat
    # descriptors start flowing ~7us earlier than 768 thin ones.
    nc = bass.Bass()
    xt_d = nc.dram_tensor("xt", [P, IC, KC, 512], BF16, kind="ExternalInput")
    wqk_d = nc.dram_tensor("wqk", [P, HPC, KC, P], BF16, kind="ExternalInput")
    bqk_d = nc.dram_tensor("bqk", [P, HPC], F32, kind="ExternalInput")
    wv_d = nc.dram_tensor("wv", [P, KC, DV], BF16, kind="ExternalInput")
    wos_d = nc.dram_tensor("wos", [P, 2, D], BF16, kind="ExternalInput")
    mask_d = nc.dram_tensor("mask", [P, 4, 2, 512], BF16, kind="ExternalInput")
    out_d = nc.dram_tensor("out", [S, D], BF16, kind="ExternalOutput")
    scratch_d = nc.dram_tensor("scratch", [HD + 1, 512], F32)
    gate_d = nc.dram_tensor("gatescr", [1, 8], BF16)

    with tile.TileContext(nc) as tc:
        with (
            tc.tile_pool(name="const", bufs=1) as cp,
            tc.tile_pool(name="exp", bufs=40) as exp_p,
            tc.tile_pool(name="zr", bufs=2) as zr_p,
            tc.tile_pool(name="outp", bufs=2) as op,
            tc.tile_pool(name="proj", bufs=2, space="PSUM") as proj_p,
            tc.tile_pool(name="scps", bufs=2, space="PSUM") as sc_p,
            tc.tile_pool(name="avps", bufs=2, space="PSUM") as av_p,
        ):
            # ---- PE warm-up source via DVE memset (gpsimd starts slowly) ----
            warm_src = cp.tile([P, 520], BF16)
            nc.vector.memset(warm_src, 1.0)

            # ---- resident SBUF tensors ----
            # The 16 hw DMA engines are a shared ~250GB/s pool: concurrent
            # transfers steal bandwidth from each other, so the critical
            # first-projection loads (xt chunk 0 + wqk) are issued alone;
            # the bulk (mask/wv/wos, xt chunks 1+2) is gated behind tiny
            # compute ops that only unblock once the first projections are
            # evacuating (~13us), keeping the early window clean.
            wqk_sb = cp.tile([P, HPC, KC, P], BF16)
            xt_sb = cp.tile([P, IC, KC, 512], BF16)
            bqk_sb = cp.tile([P, HPC], F32)
            wv_sb = cp.tile([P, KC, DV], BF16)
            mask_sb = cp.tile([P, 4, 2, 512], BF16)
            wos_sb = cp.tile([P, 2, D], BF16)

            # The DMA engines round-robin among ALL active transfers
            # (~250GB/s shared, data flow starts ~8.5us in), so issue the
            # early-needed set together and gate only the genuinely
            # late-needed bulk (xt chunk 2, mask, wv, wos) behind compute.
            nc.sync.dma_start(wqk_sb[:, 0], wqk_d[:, 0])
            nc.sync.dma_start(xt_sb[:, 0], xt_d[:, 0])
            nc.sync.dma_start(wqk_sb[:, 1:3], wqk_d[:, 1:3])
            nc.sync.dma_start(xt_sb[:, 3], xt_d[:, 3])
            nc.sync.dma_start(bqk_sb, bqk_d[:])
            nc.sync.dma_start(xt_sb[:, 1], xt_d[:, 1])

            # V with ones columns HD:P: every AV psum row 64:128 then
            # accumulates the softmax denominator Z, pre-broadcast, and the
            # AV matmul gets a full 128-wide stationary operand. Cols 0:HD
            # are overwritten by the V-projection evacuations.
            v1 = cp.tile([P, NIO, HPC, P], BF16)
            nc.vector.memset(v1[:, :, :, HD:P], 1.0)

            # pair-stacked Q^T/K^T for heads 0,1; head 2's K/Q duplicated
            # into both partition halves so its scores pair on T0/T8 too
            qTp = cp.tile([P, S], BF16)
            klp = cp.tile([P, S], BF16)
            qT2d = cp.tile([P, S], BF16)
            kl2d = cp.tile([P, S], BF16)
            # ctx for h0/h1 stacked on partitions: the output projection
            # contracts both heads in one K=128 matmul; h2 separate (K=64)
            ctxT01 = cp.tile([P, S], BF16)
            ctxT2 = cp.tile([HD, S], BF16)

            # ---- PE warm-up: (128,128)-mode matmuls ----
            # Just enough to bridge until the first projection's DMA lands
            # (~6us); the projections themselves sustain the HAM ramp.
            warm_ps = av_p.tile([P, 512], F32, tag="av", name="warm")
            for w in range(5):
                nc.tensor.matmul(
                    warm_ps[0 : HD + 1, :],
                    lhsT=warm_src[:, 0:65],
                    rhs=warm_src[:, 0:512],
                    start=True,
                    stop=(w == 4),
                )
            warm_sb = zr_p.tile([HD + 1, 512], F32, tag="warm", name="warmsb")
            nc.vector.tensor_copy(warm_sb, warm_ps[0 : HD + 1, :])
            nc.sync.dma_start(scratch_d[:], warm_sb)

            carry = {}   # ic -> (pair ex tiles, h2 ex tiles)

            def proj_slice(ic, sl):
                # wqk slices: 0 -> [K_h0|K_h1], 1 -> [K_h2|Q_h2],
                #             2 -> [Q_h0|Q_h1]
                isl = ts(ic, 512)
                ps = proj_p.tile([P, 512], F32, tag="proj")
                for kc in range(KC):
                    nc.tensor.matmul(
                        ps,
                        lhsT=wqk_sb[:, sl, kc, :],
                        rhs=xt_sb[:, ic, kc, :],
                        start=(kc == 0),
                        stop=(kc == KC - 1),
                    )
                if sl == 0:
                    nc.vector.tensor_scalar(
                        klp[:, isl], ps, bqk_sb[:, 0:1], None, ADD,
                    )
                elif sl == 1:
                    # duplicate K_h2/Q_h2 into both partition halves
                    nc.vector.tensor_scalar(
                        kl2d[0:HD, isl], ps[0:HD, :], bqk_sb[0:HD, 1:2],
                        None, ADD,
                    )
                    nc.vector.tensor_scalar(
                        kl2d[HD:P, isl], ps[0:HD, :], bqk_sb[0:HD, 1:2],
                        None, ADD,
                    )
                    nc.vector.tensor_scalar(
                        qT2d[0:HD, isl], ps[HD:P, :], bqk_sb[HD:P, 1:2],
                        None, ADD,
                    )
                    nc.vector.tensor_scalar(
                        qT2d[HD:P, isl], ps[HD:P, :], bqk_sb[HD:P, 1:2],
                        None, ADD,
                    )
                else:
                    nc.vector.tensor_scalar(
                        qTp[:, isl], ps, bqk_sb[:, 2:3], None, ADD,
                    )

            def trim_of(jc, ic):
                koff = jc - 4 * ic
                return P * koff if koff > 0 else 0

            def sc_group_pair(ic, jc):
                t = trim_of(jc, ic)
                koff = jc - 4 * ic
                sc = sc_p.tile([P, 2, 512], F32, tag="sc", name=f"sp{ic}_{jc}")
                for h in range(2):
                    hsl = ts(h, HD)
                    nc.tensor.matmul(
                        sc[:, h, t:],
                        lhsT=klp[hsl, ts(jc, P)],
                        rhs=qTp[hsl, ic * 512 + t : (ic + 1) * 512],
                        start=True,
                        stop=True,
                    )
                ex = exp_p.tile([P, 2, 512], BF16, tag="ex", name=f"xp{ic}_{jc}")
                nc.scalar.activation(ex[:, :, t:], sc[:, :, t:], AF.Exp)
                if koff >= 0:
                    nc.gpsimd.tensor_tensor(
                        ex[:, :, t:], ex[:, :, t:],
                        mask_sb[:, koff, :, t:], MUL,
                    )
                carry[ic][0].append(ex)

            def sc_group_h2(ic, jb):
                # the two key-blocks go to T0/T8 concurrently via the
                # duplicated partition halves of kl2d/qT2d
                sc = sc_p.tile([P, 2, 512], F32, tag="sc", name=f"s2_{ic}_{jb}")
                for k in range(2):
                    jc = jb + k
                    t = trim_of(jc, ic)
                    hs = slice(k * HD, (k + 1) * HD)
                    nc.tensor.matmul(
                        sc[:, k, t:],
                        lhsT=kl2d[hs, ts(jc, P)],
                        rhs=qT2d[hs, ic * 512 + t : (ic + 1) * 512],
                        start=True,
                        stop=True,
                    )
                ex = exp_p.tile([P, 2, 512], BF16, tag="ex", name=f"x2_{ic}_{jb}")
                koff = jb - 4 * ic
                if koff >= 0 and trim_of(jb + 1, ic) > 0:
                    for k in range(2):
                        t = trim_of(jb + k, ic)
                        nc.scalar.activation(ex[:, k, t:], sc[:, k, t:], AF.Exp)
                        nc.gpsimd.tensor_tensor(
                            ex[:, k, t:], ex[:, k, t:],
                            mask_sb[:, koff + k, 0, t:], MUL,
                        )
                else:
                    nc.scalar.activation(ex, sc, AF.Exp)
                    if koff >= 0:
                        nc.gpsimd.tensor_tensor(
                            ex, ex, mask_sb[:, koff : koff + 2, 0, :], MUL,
                        )
                carry[ic][1].append(ex)

            # z chains, fully on ACT: the AV psum rows HD:P already hold Z
            # pre-broadcast (ones columns of v1), so 1/Z = exp(-ln Z) is
            # two partition-parallel ACT table ops, no DMA, no PE.
            # (custom-DVE reciprocal_approx_fast would be cheaper still but
            # the axon compile path can't emit custom DVE ISA ops.)
            zbs = {}    # (ic, h) -> zb tile

            def make_zchain_a(ic, h, state):
                def go():
                    lnb = zr_p.tile([HD, 512], F32, tag="lnz",
                                    name=f"ln{ic}{h}")
                    nc.scalar.activation(lnb, state[h][HD:P, :], AF.Ln)
                    zbs[(ic, h, "ln")] = lnb
                return go

            def make_zchain_b(ic, h, state):
                def go():
                    lnb = zbs.pop((ic, h, "ln"))
                    zb = zr_p.tile([HD, 512], F32, tag="zb", name=f"zb{ic}{h}")
                    nc.scalar.activation(zb, lnb, AF.Exp, scale=-1.0)
                    zbs[(ic, h)] = zb
                return go

            def make_mult(ic, h, state, parts=1):
                def go():
                    avt = state.pop(h)
                    zb = zbs.pop((ic, h))
                    w = 512 // parts
                    for c in range(parts):
                        cs = slice(c * w, (c + 1) * w)
                        osl = slice(ic * 512 + c * w, ic * 512 + (c + 1) * w)
                        if h == 0:
                            out = ctxT01[0:HD, osl]
                        elif h == 1:
                            out = ctxT01[HD:P, osl]
                        else:
                            out = ctxT2[:, osl]
                        nc.vector.tensor_tensor(
                            out, avt[0:HD, cs], zb[:, cs], MUL,
                        )
                return go

            def av_stream(ic):
                # mm batches + fin, with each head's mult deferred one head
                n_j = 4 * ic + 4
                state = {}
                work = []

                def mk_mm(h, j0, j1):
                    def go():
                        if h not in state:
                            if ic == 0 and h == 2:
                                t_ = sc_p.tile([P, 2, 512], F32, tag="sc",
                                               name=f"av{ic}{h}")
                                state[h] = t_[:, 0, :]
                            else:
                                state[h] = av_p.tile(
                                    [P, 512], F32, tag="av", name=f"av{ic}{h}"
                                )
                        avt = state[h]
                        exs, exs2 = carry[ic]
                        for jc in range(j0, j1):
                            t = trim_of(jc, ic)
                            exap = (exs[jc][:, h, t:] if h < 2
                                    else exs2[jc // 2][:, jc % 2, t:])
                            nc.tensor.matmul(
                                avt[:, t:],
                                lhsT=v1[:, jc, h, :],
                                rhs=exap,
                                start=(jc == 0),
                                stop=(jc == n_j - 1),
                            )
                    return go

                if ic == 0:
                    # tail chunk: h2's AV accumulates in a free sc-pool bank,
                    # so all three AVs and z-chains overlap before any mult.
                    # Mults split 2x256 so oproj(0) unblocks per half-chunk.
                    for h in range(HPC):
                        for j0 in range(0, n_j, 2):
                            work.append(mk_mm(h, j0, min(j0 + 2, n_j)))
                        work.append(make_zchain_a(ic, h, state))
                        work.append(make_zchain_b(ic, h, state))
                    for h in range(HPC):
                        work.append(make_mult(ic, h, state, parts=2))
                    return work
                for h in range(HPC):
                    # batches of 2 key-chunks with a dummy before every
                    # batch: in the ACT-bound phases the PE inevitably
                    # waits on the live exp stream, and the dummies turn
                    # that wait into HAM-visible activity (idle epochs
                    # halve the duty cycle for 10-30us).
                    mms = [mk_mm(h, j0, min(j0 + 2, n_j))
                           for j0 in range(0, n_j, 2)]
                    items = []
                    for i, m in enumerate(mms):
                        if i >= len(mms) - 2:
                            items.append(dummy128)
                        items.append(m)
                    if h >= 1:
                        items.insert(1, make_zchain_b(ic, h - 1, state))
                        items.insert(2, make_mult(ic, h - 1, state))
                    items.append(make_zchain_a(ic, h, state))
                    work.extend(items)
                work.append(make_zchain_b(ic, HPC - 1, state))
                work.append(make_mult(ic, HPC - 1, state))
                return work

            out_r = out_d.rearrange("(io p) d -> p io d", p=P)
            opair = {}

            def oproj_units(ic, finegrain=False):
                # units in io pairs sharing one o_sb tile and one output DMA;
                # finegrain (tail chunk): one DMA per io on rotating queues
                # so the last write isn't one big serialized transfer.
                units = []
                dma_engines = (nc.sync, nc.gpsimd, nc.scalar, nc.sync)
                for io4 in range(4):
                    io = ic * 4 + io4

                    def unit(io=io, io4=io4):
                        if io % 2 == 0:
                            opair[io // 2] = op.tile(
                                [P, 2, D], BF16, tag="osb", name=f"ou{io}"
                            )
                        o_sb = opair[io // 2]
                        for ot, ow in ((0, 512), (1, 256)):
                            ps = proj_p.tile([P, 512], F32, tag="proj")
                            pso = ps[:, :ow]
                            osl = slice(ot * 512, ot * 512 + ow)
                            nc.tensor.matmul(
                                pso, lhsT=ctxT01[:, ts(io, P)],
                                rhs=wos_sb[:, 0, osl],
                                start=True, stop=False,
                            )
                            nc.tensor.matmul(
                                pso, lhsT=ctxT2[:, ts(io, P)],
                                rhs=wos_sb[0:HD, 1, osl],
                                start=False, stop=True,
                            )
                            nc.vector.tensor_copy(
                                o_sb[:, io % 2, osl], pso
                            )
                        if finegrain:
                            dma_engines[io4].dma_start(
                                out_r[:, io : io + 1, :],
                                o_sb[:, io % 2 : io % 2 + 1, :],
                            )
                            if io % 2 == 1:
                                opair.pop(io // 2)
                        elif io % 2 == 1:
                            nc.sync.dma_start(
                                out_r[:, io - 1 : io + 1, :],
                                opair.pop(io // 2),
                            )

                    units.append(unit)
                return units

            def v_units(ic):
                units = []
                for io in range(ic * 4, ic * 4 + 4):
                    def unit(io=io):
                        ps = proj_p.tile([P, 512], F32, tag="proj")
                        psv = ps[:, :DV]
                        c0 = (io % 4) * P
                        for kc in range(KC):
                            nc.tensor.matmul(
                                psv,
                                lhsT=xt_sb[:, io // 4, kc, c0 : c0 + P],
                                rhs=wv_sb[:, kc, :],
                                start=(kc == 0),
                                stop=(kc == KC - 1),
                            )
                        nc.vector.tensor_copy(
                            v1[:, io, :, 0:HD],
                            psv.rearrange("p (h e) -> p h e", e=HD),
                        )
                    units.append(unit)
                return units

            # PE fillers for HAM: bare weight loads stream the array for
            # ~53ns each with NO psum write, so they never churn a tile
            # ring or create WAR stalls — pure duty-cycle padding.
            def ldw_fill(n=4):
                for _ in range(n):
                    nc.tensor.ldweights(warm_src[:, 0:P])

            # dummy matmuls keep PE activity above HAM's re-throttle window
            # through the sparse tail (they write scratch psum, never read)
            def dummy64():
                d = sc_p.tile([P, 2, 512], F32, tag="sc", name="dm64")
                nc.tensor.matmul(
                    d[0:65, 0, :], lhsT=warm_src[0:HD, 0:65],
                    rhs=warm_src[0:HD, 0:512], start=True, stop=True,
                )
                return d

            def dummy128():
                d = proj_p.tile([P, 512], F32, tag="proj", name="dm128")
                nc.tensor.matmul(
                    d, lhsT=warm_src[:, 0:P],
                    rhs=warm_src[:, 0:512], start=True, stop=True,
                )
                return d

            def run_phase(ic, work128, work64=(), keep_warm=False,
                          h2_delay=False, shift=0):
                # Emit scores for chunk ic in jb-slots; after each slot emit a
                # proportional share of 64-mode fillers (oproj) and 128-mode
                # work (V proj or AV of the larger chunk). shift=1 delays the
                # 128-mode stream by one slot so AV consumers trail the live
                # exp stream by a full slot of margin.
                carry[ic] = ([], [])
                n_j = 4 * ic + 4
                work64 = list(work64)
                work128 = list(work128)
                nslots = n_j // 2
                d64 = d128 = 0
                pend_h2 = None
                for s, jb in enumerate(range(0, n_j, 2)):
                    sc_group_pair(ic, jb)
                    sc_group_pair(ic, jb + 1)
                    if h2_delay:
                        if pend_h2 is not None:
                            sc_group_h2(ic, pend_h2)
                        pend_h2 = jb
                    else:
                        sc_group_h2(ic, jb)
                    w = len(work64) * (s + 1) // nslots
                    while d64 < w:
                        work64[d64]()
                        d64 += 1
                    if keep_warm:
                        dummy64()
                    w = len(work128) * max(0, s + 1 - shift) // nslots
                    while d128 < w:
                        work128[d128]()
                        d128 += 1
                    if keep_warm:
                        dummy128()
                if pend_h2 is not None:
                    sc_group_h2(ic, pend_h2)
                while d128 < len(work128):
                    work128[d128]()
                    d128 += 1

            # ---------------- main schedule ----------------
            # K(0) + Q(3) first so scores(3) -- and the ACT exp stream --
            # start as early as possible; the remaining K slices drip in as
            # 128-mode fillers just ahead of the key chunks that need them.
            # V/Q projections of the small chunks are pushed late to densify
            # the back half of the kernel (keeps HAM un-throttled).
            proj_slice(0, 0)   # K-pair(0): pair scores jc 0-3
            # bulk loads, gated on the first projections' evacuations so
            # they don't steal DMA bandwidth from the critical early loads
            gate_s = zr_p.tile([1, 8], BF16, tag="gate", name="gate_s")
            nc.scalar.activation(gate_s, klp[0:1, 0:8], AF.Copy)
            nc.scalar.dma_start(wv_sb, wv_d[:])
            nc.scalar.dma_start(wos_sb, wos_d[:])
            proj_slice(3, 2)   # Q-pair(3)
            gate_g = zr_p.tile([1, 8], BF16, tag="gate", name="gate_g")
            nc.gpsimd.tensor_tensor(
                gate_g, qTp[0:1, 1536:1544], qTp[0:1, 1536:1544], MUL
            )
            nc.gpsimd.dma_start(xt_sb[:, 2], xt_d[:, 2])
            nc.gpsimd.dma_start(mask_sb, mask_d[:])
            kdrip = [lambda: proj_slice(0, 1), lambda: proj_slice(3, 1)]
            for ic in (1, 2):
                kdrip.append(lambda ic=ic: proj_slice(ic, 0))
                kdrip.append(lambda ic=ic: proj_slice(ic, 1))
            kdrip.append(lambda: proj_slice(3, 0))
            run_phase(3, kdrip + [lambda: proj_slice(2, 2)]
                      + v_units(3) + v_units(2) + v_units(1) + v_units(0),
                      h2_delay=True)
            run_phase(2, av_stream(3) + [lambda: proj_slice(1, 2)], shift=1)
            run_phase(1, av_stream(2) + [lambda: proj_slice(0, 2)],
                      oproj_units(3), keep_warm=True, shift=1)
            run_phase(0, av_stream(1), oproj_units(2), keep_warm=True,
                      shift=1)
            # tail: oproj(1) units interleaved with av(0) + z-chains so the
            # PE stays dense while the (fast, DMA-free) z-chains resolve;
            # then the mults and chunk-0 output projections.
            av0 = av_stream(0)
            o1 = oproj_units(1)
            tail = [av0[0], av0[1], av0[2], o1[0], av0[3], av0[4], dummy128,
                    av0[5], av0[6], o1[1], av0[7], av0[12], dummy128,
                    av0[8], dummy128, av0[9], av0[10], o1[2], av0[11],
                    av0[13], o1[3], av0[14]]
            for item in tail:
                item()
            d64 = None
            for u in oproj_units(0, finegrain=True):
                u()
                d64 = dummy64()
            drain = zr_p.tile([HD, 512], F32, tag="ddr", name="ddrain")
            if d64 is not None:
                nc.vector.tensor_copy(drain, d64[0:HD, 0, :])

    _split_excess_waits(nc)
    return nc


_NC = None


def _get_nc():
    global _NC
    if _NC is None:
        _NC = _build_module()
    return _NC


def _make_mask():
    p = np.arange(P)[:, None]
    f = np.arange(512)[None, :]
    m = np.empty((P, 4, 2, 512), np.float32)
    for k in range(4):
        m[:, k, 0, :] = (p <= f - P * k).astype(np.float32)
        m[:, k, 1, :] = m[:, k, 0, :]
    return m.astype(NPBF16)


def _build_in_maps(x, wq, bq, wk, bk, wv, bv, wo):
    scale = 1.0 / np.sqrt(HD)
    mask = _make_mask()
    in_maps = []
    for core in range(8):
        b = core // 4
        h0 = (core % 4) * HPC

        # pair-packed slices: [K_h0|K_h1], [K_h2|Q_h2], [Q_h0|Q_h1]
        wqk = np.empty((D, DQK), np.float32)
        bqk = np.empty((P, HPC), np.float32)
        cs = [slice((h0 + i) * HD, (h0 + i + 1) * HD) for i in range(HPC)]
        wqk[:, 0:HD] = wk[:, cs[0]]
        wqk[:, HD:P] = wk[:, cs[1]]
        wqk[:, P : P + HD] = wk[:, cs[2]]
        wqk[:, P + HD : 2 * P] = wq[:, cs[2]] * scale
        wqk[:, 2 * P : 2 * P + HD] = wq[:, cs[0]] * scale
        wqk[:, 2 * P + HD : 3 * P] = wq[:, cs[1]] * scale
        bqk[0:HD, 0] = bk[cs[0]]
        bqk[HD:P, 0] = bk[cs[1]]
        bqk[0:HD, 1] = bk[cs[2]]
        bqk[HD:P, 1] = bq[cs[2]] * scale
        bqk[0:HD, 2] = bq[cs[0]] * scale
        bqk[HD:P, 2] = bq[cs[1]] * scale

        vcols = slice(h0 * HD, (h0 + HPC) * HD)
        # wos: [P, 2, D] with h0/h1 stacked in slot 0 (K=128 oproj matmul)
        # and h2 in rows 0:64 of slot 1
        w_all = wo[vcols, :].reshape(HPC, HD, D)
        wos = np.zeros((P, 2, D), np.float32)
        wos[0:HD, 0] = w_all[0]
        wos[HD:P, 0] = w_all[1]
        wos[0:HD, 1] = w_all[2]

        # pre-rearranged for one-descriptor-per-partition DMA loads
        xt = (
            x[b].T.reshape(KC, P, IC, 512).transpose(1, 2, 0, 3)
        )  # [P, IC, KC, 512]
        wqk_r = wqk.reshape(KC, P, HPC, P).transpose(1, 2, 0, 3)
        wv_r = wv[:, vcols].reshape(KC, P, DV).transpose(1, 0, 2)

        in_maps.append(
            {
                "xt": np.ascontiguousarray(xt).astype(NPBF16),
                "wqk": np.ascontiguousarray(wqk_r).astype(NPBF16),
                "bqk": bqk.astype(np.float32),
                "wv": np.ascontiguousarray(wv_r).astype(NPBF16),
                "wos": np.ascontiguousarray(wos).astype(NPBF16),
                "mask": mask,
            }
        )
    return in_maps


def kernel(x, wq, bq, wk, bk, wv, bv, wo, bo):
    x = np.asarray(x, np.float32)
    wq = np.asarray(wq, np.float32)
    bq = np.asarray(bq, np.float32)
    wk = np.asarray(wk, np.float32)
    bk = np.asarray(bk, np.float32)
    wv = np.asarray(wv, np.float32)
    bv = np.asarray(bv, np.float32)
    wo = np.asarray(wo, np.float32)
    bo = np.asarray(bo, np.float32)

    in_maps = _build_in_maps(x, wq, bq, wk, bk, wv, bv, wo)
    res = run_bass_kernel_spmd(_get_nc(), in_maps, core_ids=list(range(8)))
    out = np.zeros((B, S, D), np.float32)
    for core in range(8):
        out[core // 4] += np.asarray(res.results[core]["out"], np.float32)
    out += bo + bv @ wo
    return out

